# revision 9
# baseline (speedup 1.0000x reference)
"""CostDifference kernel for Trainium2 (Bass/Tile), 8-core SPMD.

out[n, d, c, h, w] = left[n,c,h,w] - right[n,c,h+s,w] for h+s < H else 0,
where s = 128 - d (disparities d = 0..127 <-> shifts s = 128..1).

Sharding: channel-parallel. Core k handles channels {2k, 2k+1} and ALL 128
disparities, so the Bass program is identical on every core (AP shapes and
offsets are compile-time constants shared by all cores) and only the input
data differs. Output per core: [128, 2, 128, 256] (32 MiB), gathered on the
host by concatenation along the channel axis.

On-chip layout: H on partitions, (c, w) on the free axis. The per-disparity
partition shift is absorbed by the HBM->SBUF load DMA (DMA may place rows at
any partition offset; compute engines may not). 4 disparities are merged per
DVE tensor_sub by stacking them in the free dimension (free size 4*512=2048),
which amortizes the per-instruction overhead.

Zero rows (h >= d) are never written: run_bass_kernel_spmd pre-zeroes
ExternalOutput buffers (native path) / donates zero buffers (PJRT path), a
documented contract kernels may rely on.
"""

import os
import sys

sys.path.insert(0, "/opt/trn_rl_repo")

import numpy as np

import concourse.bacc as bacc
from concourse.bass import AP
import concourse.mybir as mybir
from concourse import tile
from concourse.bass_utils import run_bass_kernel_spmd

N, C, H, W = 1, 16, 128, 256
D = 128                      # disparities; d has shift s = 128 - d
N_CORES = 8
C_LOC = C // N_CORES         # channels per core
FREE = C_LOC * W             # free elems per disparity chunk (512)
QUAD = int(os.environ.get("K_QUAD", "4"))   # disparities merged per DVE op
N_BUFS = int(os.environ.get("K_BUFS", "4"))
PAD = QUAD - 1               # zero rows appended to right (uniform quad loads)
_SKIP = os.environ.get("K_SKIP", "")        # bench-only: "loads","stores","sub"

_cached = {}


def _build_program():
    f32 = mybir.dt.float32
    nc = bacc.Bacc("TRN2", target_bir_lowering=False, debug=False,
                   enable_asserts=False, num_devices=N_CORES)
    # all DRAM tensors h-major with (c, w) flattened: 2 KiB contiguous runs
    left_h = nc.dram_tensor("left", [H, FREE], f32, kind="ExternalInput")
    right_h = nc.dram_tensor("right", [H + PAD, FREE], f32,
                             kind="ExternalInput")
    out_h = nc.dram_tensor("out", [D, H, FREE], f32, kind="ExternalOutput")

    with tile.TileContext(nc) as tc:
        with tc.tile_pool(name="sbuf", bufs=1) as pool:
            # left replicated QUAD times along free dim: [h, quad*(c,w)]
            lq = pool.tile([H, QUAD * FREE], f32, tag="lq")
            for q in range(QUAD):
                nc.sync.dma_start(
                    out=lq[:, q * FREE:(q + 1) * FREE], in_=left_h[:])
            rr_tiles = []
            oq_tiles = []
            for b in range(N_BUFS):
                rt = pool.tile([H, QUAD * FREE], f32, name=f"rr{b}", tag=f"rr{b}")
                nc.vector.memset(rt[:], 0.0)
                rr_tiles.append(rt)
                oq_tiles.append(pool.tile([H, QUAD * FREE], f32,
                                          name=f"oq{b}", tag=f"oq{b}"))

            rings = [nc.sync, nc.scalar]  # the two HWDGE FIFO rings
            for qi in range(D // QUAD):
                rr = rr_tiles[qi % N_BUFS]
                oq = oq_tiles[qi % N_BUFS]
                d_hi = qi * QUAD + QUAD - 1
                # chunk j' holds disparity d = d_hi - j' (reversed so the
                # DRAM-side j' stride is +W); one 4D DMA loads the whole quad:
                # rr[h, j', c, w] <- right_pad[c, (128 - d_hi) + h + j', w].
                # Rows past H read host-appended zeros.
                if "loads" not in _SKIP:
                    rings[qi % 2].dma_start(
                        out=rr[0:d_hi, :].rearrange("p (j f) -> p j f", j=QUAD),
                        in_=AP(right_h, (D - d_hi) * FREE,
                               [[FREE, d_hi], [FREE, QUAD], [1, FREE]]),
                    )
                if "sub" not in _SKIP:
                    nc.vector.tensor_sub(
                        out=oq[0:d_hi, :], in0=lq[0:d_hi, :], in1=rr[0:d_hi, :])
                if "stores" not in _SKIP:
                    for j in range(QUAD):
                        d = qi * QUAD + j
                        if d == 0:
                            continue
                        jc = d_hi - d  # chunk index for disparity d
                        rings[d % 2].dma_start(
                            out=out_h[d, 0:d, :],
                            in_=oq[0:d, jc * FREE:(jc + 1) * FREE],
                        )
    nc.compile()
    return nc


def _run(left, right, trace=False):
    """left/right: [N, C, H, W] f32. Returns (full_out, exec_time_ns)."""
    if "nc" not in _cached:
        _cached["nc"] = _build_program()
    nc = _cached["nc"]
    left = np.ascontiguousarray(np.asarray(left), dtype=np.float32)
    right = np.ascontiguousarray(np.asarray(right), dtype=np.float32)
    in_maps = []
    for k in range(N_CORES):
        sl = slice(k * C_LOC, (k + 1) * C_LOC)
        lt = left[0, sl].transpose(1, 0, 2).reshape(H, FREE)
        rt = right[0, sl].transpose(1, 0, 2).reshape(H, FREE)
        rp = np.concatenate([rt, np.zeros((PAD, FREE), np.float32)], axis=0)
        in_maps.append({
            "left": np.ascontiguousarray(lt),
            "right": np.ascontiguousarray(rp),
        })
    res = run_bass_kernel_spmd(nc, in_maps, core_ids=list(range(N_CORES)),
                               trace=trace)
    # results[k]["out"]: [D, H, C_LOC*W] -> [D, C_LOC, H, W], concat channels
    parts = [
        res.results[k]["out"].reshape(D, H, C_LOC, W).transpose(0, 2, 1, 3)
        for k in range(N_CORES)
    ]
    full = np.concatenate(parts, axis=1)
    return np.ascontiguousarray(full[None]), res.exec_time_ns


def kernel(left, right):
    out, _ = _run(left, right, trace=False)
    return out
